# revision 1
# baseline (speedup 1.0000x reference)
"""GAT layer (PyG-style, add_self_loops=True) on 8 Trainium2 NeuronCores.

Strategy (per sharding hint): partition destination nodes (and their incident
edges) across the 8 cores; each core owns a contiguous range of 6250 dst nodes.

Per core:
  phase 1: full projection table row[n] = [h(256) | asr(4)] in bf16 (768-B
           pitch, 520 B written) in local DRAM -- replicated compute, zero
           cross-core communication.  Split into TWO tables (lo: nodes <
           25088, hi: rest) because dma_gather indices are int16.
           All projection matmuls run in bf16 (x is pre-cast on host).
  phase 1b: a_dst (bf16) for the core's own 6272 dst nodes, kept in SBUF
           (variant a) or written as a compact 256-B-row DRAM table for a
           third gather (variant b).
  phase 2 (variant a, default): per window of 128 dst nodes, two dma_gathers
           (lo/hi) pull the 768-B source rows for all incident edges.  The
           edge->dst-slot one-hot (and its transpose) arrive from the host as
           packed int16 BITMASKS in bit-plane order and are expanded to bf16
           0/1 matrices with whole-window DVE ops (bitwise_and + is_equal,
           all APs stride-1 so the DVE 2x mode engages) -- no PE transposes.
           a_dst per edge = kj tiny bf16 matmuls (lhsT = one-hot transpose,
           rhs = adst).  scores: sc = asr + adx (both single bf16 -- the
           hi/lo split-precision was dropped, measured error budget allows
           it); e = max(exp(sc), exp(0.2*sc)) (== exp(leaky_relu(sc)),
           exp on Act with its scale port, max on DVE writing bf16 straight
           into the message tile).  messages mv = [e*h | e] bf16 with the
           table in (d,h)-major layout so the e-broadcast multiply stays
           stride-1; segment-sum = kj bf16 matmuls accumulating into one
           PSUM tile; finalize on DVE, out-DMA on the Act HWDGE ring (keeps
           the SP ring a pure load stream).  Gathers for window w+2 are
           issued before computing window w, and the lo-table gathers begin
           while the hi half of the table is still being built.
           Softmax max-subtraction is skipped (shift-invariant, scores O(1)).

Pad edges point at a dummy table row AND have all-zero one-hot bits, so they
contribute exactly nothing.

Host does only index-space work (self-loop append, dst sort, windowing,
padding, int16 index wrapping, one-hot bit packing) plus data layout
(x transposed + cast bf16).
"""

import math

import numpy as np

N = 50000
IN_DIM = 64
H = 4
D = 64
HD = H * D  # 256
ROWC = 384  # bf16 table row pitch: h(256) | asr(4) | pad
WCOLS = HD + H  # 260 cols actually written per table row: h | asr
NEG_SLOPE = 0.2
EPS = 1e-16

NCORES = 8
NPC = N // NCORES  # 6250 dst nodes per core
NWIN = math.ceil(NPC / 128)  # 49 windows
WROWS = NWIN * 128  # 6272
NT1 = 392  # phase-1 tiles (50176 nodes incl. pad)
NROWS_ALL = NT1 * 128  # 50176
SPLIT_T = 196  # lo/hi table split, in 128-row tiles
SPLIT = SPLIT_T * 128  # 25088
LO_TILES = SPLIT_T + 1  # +1 dummy tile
LO_ROWS = LO_TILES * 128  # 25216
HI_TILES = NT1 - SPLIT_T  # 196
HI_ROWS = HI_TILES * 128  # 25088
DUMMY_LO = SPLIT  # row 25088 of lo table (dedicated dummy row)
DUMMY_HI = N - SPLIT  # row 24912 of hi table (= node 50000, h == 0)
B1 = 7  # phase-1 tiles per iteration (divides both 196 and 392)
DSTC = 128  # bf16 cols per tbl_dst row (256 B): adst(4) | pad

LAST_RESULTS = None  # BassKernelResults of the most recent run (for test.py)


def _wrap_idx(ids):
    """[n] int -> dma_gather wrapped layout [128, n/16] int16
    (idx i at [i%16, i//16], replicated across the 8 Q7 core groups)."""
    n = len(ids)
    w16 = ids.reshape(n // 16, 16).T.astype(np.int16)  # [16, n/16]
    return np.tile(w16, (8, 1))


def _prep_host(edge_index, variant="b"):
    """Returns dict of per-core host tensors + (KL, KH)."""
    src = np.concatenate([edge_index[0], np.arange(N, dtype=np.int64)]).astype(np.int64)
    dst = np.concatenate([edge_index[1], np.arange(N, dtype=np.int64)]).astype(np.int64)
    order = np.argsort(dst, kind="stable")
    src = src[order].astype(np.int32)
    dst = dst[order].astype(np.int32)

    bounds = [c * NPC + w * 128 for c in range(NCORES) for w in range(NWIN)]
    bounds.append(N)
    cuts = np.searchsorted(dst, np.asarray(bounds))

    lo_counts = np.zeros(NCORES * NWIN, np.int64)
    hi_counts = np.zeros(NCORES * NWIN, np.int64)
    for b in range(NCORES * NWIN):
        s = src[cuts[b] : cuts[b + 1]]
        lo_counts[b] = int((s < SPLIT).sum())
        hi_counts[b] = len(s) - lo_counts[b]
    KL = max(1, math.ceil(lo_counts.max() / 128))
    KH = max(1, math.ceil(hi_counts.max() / 128))
    kj = KL + KH

    # windows 0..1 gather the full padded lists (dummy rows) so both g
    # buffers are fully initialized; later windows use -1 trailing pads +
    # true counts, and their skipped regions read the previous window's
    # (finite) data, which the all-zero one-hot columns nullify.
    ilow = np.full((NCORES, NWIN, KL * 128), DUMMY_LO, np.int32)
    ihigh = np.full((NCORES, NWIN, KH * 128), DUMMY_HI, np.int32)
    idst = np.zeros((NCORES, NWIN, kj * 128), np.int32)  # pad -> row 0
    ohbits = np.zeros((NCORES, NWIN, 128, kj * 8), np.uint16)
    ohTbits = np.zeros((NCORES, NWIN, 128, kj * 8), np.uint16)
    for c in range(NCORES):
        base = c * NPC
        for w in range(NWIN):
            b = c * NWIN + w
            s = src[cuts[b] : cuts[b + 1]]
            d = dst[cuts[b] : cuts[b + 1]] - base - w * 128
            m = s < SPLIT
            slo, dlo = s[m], d[m]
            shi, dhi = s[~m] - SPLIT, d[~m]
            # ascending source rows => HBM page locality in the gather
            o = np.argsort(slo, kind="stable")
            slo, dlo = slo[o], dlo[o]
            o = np.argsort(shi, kind="stable")
            shi, dhi = shi[o], dhi[o]
            ilow[c, w, : len(slo)] = slo
            ihigh[c, w, : len(shi)] = shi
            # negpad skip disabled: full padded gathers (dummy rows)
            # edge position in gather output: idx i -> (part i%128, subtile i//128)
            nl, nh = len(slo), len(shi)
            i = np.arange(nl)
            jl, pl = i // 128, i % 128
            i = np.arange(nh)
            jh, ph = KL + i // 128, i % 128
            jj = np.concatenate([jl, jh])
            pp = np.concatenate([pl, ph])
            ss = np.concatenate([dlo, dhi])  # dst slot per edge
            # local dst id for the a_dst gather (variant b)
            idst[c, w, jj * 128 + pp] = w * 128 + ss
            # bit-plane packing (keeps device-side expansion APs stride-1):
            # oh[p, j*128+slot]: bit (slot//8) of word [p, j*8 + slot%8]
            np.bitwise_or.at(
                ohbits[c, w], (pp, jj * 8 + ss % 8),
                (np.uint16(1) << (ss // 8).astype(np.uint16)),
            )
            # ohT[slot, j*128+p]: bit (p//8) of word [slot, j*8 + p%8]
            np.bitwise_or.at(
                ohTbits[c, w], (ss, jj * 8 + pp % 8),
                (np.uint16(1) << (pp // 8).astype(np.uint16)),
            )
    ilow_w = np.zeros((NCORES, NWIN, 128, KL * 8), np.int16)
    ihigh_w = np.zeros((NCORES, NWIN, 128, KH * 8), np.int16)
    idst_w = np.zeros((NCORES, NWIN, 128, kj * 8), np.int16)
    for c in range(NCORES):
        for w in range(NWIN):
            ilow_w[c, w] = _wrap_idx(ilow[c, w])
            ihigh_w[c, w] = _wrap_idx(ihigh[c, w])
            idst_w[c, w] = _wrap_idx(idst[c, w])
    # mask128[p, b*8+w] = 1 << b  (bit-plane expansion constant)
    mrow = (np.uint16(1) << (np.arange(128, dtype=np.uint16) // 8))
    mask128 = np.tile(mrow[None, :], (128, 1)).view(np.int16)
    return {
        "ilow": ilow_w,
        "ihigh": ihigh_w,
        "idst": idst_w,
        "ohbits": ohbits.view(np.int16),
        "ohTbits": ohTbits.view(np.int16),
        "mask16": mask128,
        "KL": KL,
        "KH": KH,
    }


def _build_program(KL, KH, variant="b", ablate="full"):
    import concourse.bass as bass
    import concourse.bacc as bacc
    import concourse.tile as tile
    from concourse import mybir

    f32 = mybir.dt.float32
    bf16 = mybir.dt.bfloat16
    i16 = mybir.dt.int16
    kj = KL + KH

    nc = bacc.Bacc(None, target_bir_lowering=False)

    xT_d = nc.dram_tensor("xT", [IN_DIM, NROWS_ALL], bf16, kind="ExternalInput")
    xdT_d = nc.dram_tensor("xdstT", [IN_DIM, WROWS], bf16, kind="ExternalInput")
    W_d = nc.dram_tensor("W", [IN_DIM, HD], f32, kind="ExternalInput")
    asrc_d = nc.dram_tensor("att_src", [1, HD], f32, kind="ExternalInput")
    adst_d = nc.dram_tensor("att_dst", [1, HD], f32, kind="ExternalInput")
    bias_d = nc.dram_tensor("bias", [1, HD], f32, kind="ExternalInput")
    il_d = nc.dram_tensor("ilow", [NWIN, 128, KL * 8], i16, kind="ExternalInput")
    ih_d = nc.dram_tensor("ihigh", [NWIN, 128, KH * 8], i16, kind="ExternalInput")
    ob_d = nc.dram_tensor("ohbits", [NWIN, 128, kj * 8], i16, kind="ExternalInput")
    if variant == "a":
        obT_d = nc.dram_tensor("ohTbits", [NWIN, 128, kj * 8], i16, kind="ExternalInput")
    else:
        id_d = nc.dram_tensor("idst", [NWIN, 128, kj * 8], i16, kind="ExternalInput")
    mask_d = nc.dram_tensor("mask16", [128, 128], i16, kind="ExternalInput")
    out_d = nc.dram_tensor("out", [WROWS, HD], f32, kind="ExternalOutput")
    tbl_lo = nc.dram_tensor("tbl_lo", [LO_ROWS, ROWC], bf16)  # 768 B pitch
    tbl_hi = nc.dram_tensor("tbl_hi", [HI_ROWS, ROWC], bf16)
    if variant == "b":
        tbl_dst = nc.dram_tensor("tbl_dst", [WROWS, DSTC], bf16)  # 256 B rows

    X = mybir.AxisListType.X
    EQ = mybir.AluOpType.is_equal
    AND = mybir.AluOpType.bitwise_and
    MULT = mybir.AluOpType.mult
    MAX = mybir.AluOpType.max

    with tile.TileContext(nc) as tc:
        with tc.tile_pool(name="const", bufs=1) as cpool:
            spsum_cm = tc.tile_pool(name="setup_psum", bufs=1, space="PSUM")
            spsum = spsum_cm.__enter__()
            ones = cpool.tile([1, 128], f32)
            nc.vector.memset(ones[:], 1.0)

            # WA = [W | Wsrc | Wdst], Wsrc[k,h] = sum_d W[k,h*D+d]*att_src[h,d]
            # built in f32, then cast to bf16 for the phase-1 matmuls.
            WACOLS = HD + 2 * H  # Wdst block feeds phase-1b only
            wa_tmp = cpool.tile([IN_DIM, WACOLS], f32)
            nc.vector.memset(wa_tmp[:], 0.0)
            nc.sync.dma_start(wa_tmp[:, 0:HD], W_d[:, :])
            att_s_raw = cpool.tile([1, HD], f32)
            nc.sync.dma_start(att_s_raw[:], asrc_d[:, :])
            att_t_raw = cpool.tile([1, HD], f32)
            nc.sync.dma_start(att_t_raw[:], adst_d[:, :])
            att_s = cpool.tile([1, HD], f32)
            nc.vector.tensor_copy(att_s[:], att_s_raw[:])
            att_t = cpool.tile([1, HD], f32)
            nc.vector.tensor_copy(att_t[:], att_t_raw[:])
            for att_tile, col0 in ((att_s, HD), (att_t, HD + H)):
                attb = spsum.tile([IN_DIM, HD], f32, tag="attb")
                nc.tensor.matmul(
                    attb[:], lhsT=ones[:1, 0:IN_DIM], rhs=att_tile[:],
                    start=True, stop=True,
                )
                tmp = cpool.tile([IN_DIM, HD], f32, tag="tmp")
                nc.vector.tensor_mul(tmp[:], wa_tmp[:, 0:HD], attb[:])
                # W and att rows arrive (d,h)-major; reduce over d
                nc.vector.reduce_sum(
                    out=wa_tmp[:, col0 : col0 + H],
                    in_=tmp[:].rearrange("k (d h) -> k h d", h=H),
                    axis=X,
                )
            WA = cpool.tile([IN_DIM, WACOLS], bf16)
            nc.vector.tensor_copy(WA[:], wa_tmp[:])

            bias_raw = cpool.tile([1, HD], f32)
            nc.sync.dma_start(bias_raw[:], bias_d[:, :])
            bias_sb = cpool.tile([1, HD], f32)
            nc.vector.tensor_copy(bias_sb[:], bias_raw[:])
            bb = spsum.tile([128, HD], f32)
            nc.tensor.matmul(bb[:], lhsT=ones[:1, :], rhs=bias_sb[:], start=True, stop=True)
            bias_bc = cpool.tile([128, HD], f32)
            nc.scalar.copy(bias_bc[:], bb[:])

            mask_bc = cpool.tile([128, 128], i16)
            mask_raw = cpool.tile([128, 128], i16, tag="mask_raw")
            nc.sync.dma_start(mask_raw[:], mask_d[:, :])
            nc.vector.tensor_copy(mask_bc[:], mask_raw[:])

            # a_dst (bf16) for the core's own dst shard: [128, NWIN*4]
            adst_all = cpool.tile([128, NWIN * H], bf16)

            spsum_cm.__exit__(None, None, None)  # free setup PSUM banks

            # ---------------- phases (unified pools, pipelined) ----------------
            with (
                tc.tile_pool(name="p1", bufs=4) as p1,
                tc.tile_pool(name="p1ps", bufs=3, space="PSUM") as p1ps,
                tc.tile_pool(name="p1psb", bufs=1, space="PSUM") as p1psb,
                tc.tile_pool(name="gat", bufs=2) as gpool,
                tc.tile_pool(name="edg", bufs=3) as epool,
                tc.tile_pool(name="wrk", bufs=2) as wpool,
                tc.tile_pool(name="fin", bufs=2) as fpool,
                tc.tile_pool(name="p2ps", bufs=3 if variant == "b" else 2,
                             space="PSUM") as p2ps,
                tc.tile_pool(name="p2psb", bufs=2, space="PSUM") as p2psb,
            ):
                # phase 1b FIRST: a_dst (hi|lo bf16) for own dst shard, so the
                # compact dst table is ready before any dst-gather fires
                for w in range(NWIN):
                    xd = p1.tile([IN_DIM, 128], bf16, tag="xdr")
                    nc.sync.dma_start(xd[:], xdT_d[:, w * 128 : (w + 1) * 128])
                    adp = p1psb.tile([128, H], f32, tag="adp")
                    nc.tensor.matmul(
                        adp[:], lhsT=xd[:], rhs=WA[:, HD + H : HD + 2 * H],
                        start=True, stop=True,
                    )
                    c0 = w * H
                    nc.vector.tensor_copy(adst_all[:, c0 : c0 + H], adp[:])
                if variant == "b":
                    nc.scalar.dma_start(
                        tbl_dst[:, 0:H].rearrange("(w p) c -> p w c", p=128),
                        adst_all[:].rearrange("p (w c) -> p w c", c=H),
                    )

                def p1_tile(it):
                    t0 = it * B1
                    xt = p1.tile([IN_DIM, B1 * 128], bf16, tag="xtr")
                    nc.sync.dma_start(xt[:], xT_d[:, t0 * 128 : (t0 + B1) * 128])
                    hs = p1.tile([128, B1 * WCOLS], bf16, tag="hs")
                    for k in range(B1):
                        hp = p1ps.tile([128, WCOLS], f32, tag="hp")
                        nc.tensor.matmul(
                            hp[:],
                            lhsT=xt[:, k * 128 : (k + 1) * 128],
                            rhs=WA[:, 0:WCOLS],
                            start=True,
                            stop=True,
                        )
                        # cast h+asr to bf16, split across DVE and Act
                        cast = nc.vector.tensor_copy if k < 3 else nc.scalar.copy
                        cast(hs[:, k * WCOLS : (k + 1) * WCOLS], hp[:])
                    if t0 < SPLIT_T:
                        dst_ap = tbl_lo[t0 * 128 : (t0 + B1) * 128, 0:WCOLS]
                    else:
                        u = t0 - SPLIT_T
                        dst_ap = tbl_hi[u * 128 : (u + B1) * 128, 0:WCOLS]
                    nc.scalar.dma_start(
                        dst_ap.rearrange("(k p) c -> p k c", p=128),
                        hs[:].rearrange("p (k c) -> p k c", c=WCOLS),
                    )

                # lo half of the src table
                for it in range(SPLIT_T // B1):
                    p1_tile(it)
                # dummy rows of the lo table (gather pads point here)
                zrow = cpool.tile([128, WCOLS], bf16)
                nc.vector.memset(zrow[:], 0.0)
                nc.vector.memset(zrow[:, HD : HD + H], -1e30)  # asr col
                nc.sync.dma_start(tbl_lo[DUMMY_LO : DUMMY_LO + 128, 0:WCOLS], zrow[:])

                # zero both g buffers once: rows skipped by the negative-pad
                # gathers then read as 0.0 (finite) forever after
                for _ in range(2):
                    gz = gpool.tile([128, kj * ROWC], bf16, tag="g")
                    nc.vector.memset(gz[:], 0.0)

                def issue_pre(w):
                    """idx DMAs + lo/dst gathers: only needs tbl_lo/tbl_dst,
                    so these run while the hi table is still being built."""
                    il_t = epool.tile([128, KL * 8], i16, tag="il")
                    nc.sync.dma_start(il_t[:], il_d[w])
                    ih_t = epool.tile([128, KH * 8], i16, tag="ih")
                    nc.sync.dma_start(ih_t[:], ih_d[w])
                    ob_t = epool.tile([128, kj * 8], i16, tag="ob")
                    nc.sync.dma_start(ob_t[:], ob_d[w])
                    g = gpool.tile([128, kj * ROWC], bf16, tag="g")
                    gv = g[:].rearrange("p (j c) -> p j c", c=ROWC)
                    nc.gpsimd.dma_gather(
                        out_ap=gv[:, 0:KL, :], in_ap=tbl_lo[:, :], idxs_ap=il_t[:],
                        num_idxs=KL * 128, num_idxs_reg=KL * 128, elem_size=ROWC,
                        single_packet=False,
                    )
                    st = {"g": g, "gv": gv, "ih": ih_t, "ob": ob_t,
                          "gdv": None, "obT": None}
                    if variant == "a":
                        obT_t = epool.tile([128, kj * 8], i16, tag="obT")
                        nc.sync.dma_start(obT_t[:], obT_d[w])
                        st["obT"] = obT_t
                    else:
                        id_t = epool.tile([128, kj * 8], i16, tag="idt")
                        nc.sync.dma_start(id_t[:], id_d[w])
                        gd = gpool.tile([128, kj * DSTC], bf16, tag="gd")
                        st["gdv"] = gd[:].rearrange("p (j c) -> p j c", c=DSTC)
                        nc.gpsimd.dma_gather(
                            out_ap=st["gdv"][:, :, :], in_ap=tbl_dst[:, :],
                            idxs_ap=id_t[:],
                            num_idxs=kj * 128, num_idxs_reg=kj * 128,
                            elem_size=DSTC, single_packet=False,
                        )
                    return st

                def issue_hi(st, w):
                    nc.gpsimd.dma_gather(
                        out_ap=st["gv"][:, KL:kj, :], in_ap=tbl_hi[:, :],
                        idxs_ap=st["ih"][:],
                        num_idxs=KH * 128, num_idxs_reg=KH * 128, elem_size=ROWC,
                        single_packet=False,
                    )

                nwin2 = NWIN if ablate != "p1" else 0
                if nwin2:
                    pend = [issue_pre(0), issue_pre(1) if nwin2 > 1 else None]

                # hi half of the src table (lo/dst gathers overlap this)
                for it in range(SPLIT_T // B1, NT1 // B1):
                    p1_tile(it)
                nc.sync.dma_start(
                    tbl_hi[DUMMY_HI : DUMMY_HI + 1, HD : HD + H],
                    zrow[:1, HD : HD + H],
                )

                if ablate == "p1":
                    zo = fpool.tile([128, HD], f32)
                    nc.vector.memset(zo[:], 0.0)
                    for w in range(NWIN):
                        nc.sync.dma_start(out_d[w * 128 : (w + 1) * 128, :], zo[:])

                for w in range(nwin2):
                    st = pend[0]
                    issue_hi(st, w)
                    pend[0] = pend[1]
                    if w + 2 < nwin2:
                        pend[1] = issue_pre(w + 2)
                    gv, gdv, ob_t, obT_t = st["gv"], st["gdv"], st["ob"], st["obT"]
                    g = st["g"]

                    if ablate == "p1g":
                        outw = fpool.tile([128, HD], f32, tag="outw")
                        nc.vector.tensor_copy(outw[:], g[:, 0:HD])
                        nc.scalar.dma_start(out_d[w * 128 : (w + 1) * 128, :], outw[:])
                        continue

                    # one-hot expand (bit-plane): bits -> bf16 0/1 [128, kj*128]
                    # col j*128 + b*8 + v <=> slot b*8+v; all APs stride-1 last
                    ohtmp = wpool.tile([128, kj * 128], i16, tag="ohtmp")
                    nc.vector.tensor_tensor(
                        out=ohtmp[:].rearrange("p (j b v) -> p j b v", b=16, v=8),
                        in0=ob_t[:].rearrange("p (j v) -> p j v", v=8)
                        .unsqueeze(2).to_broadcast([128, kj, 16, 8]),
                        in1=mask_bc[:].rearrange("p (b v) -> p b v", v=8)
                        .unsqueeze(1).to_broadcast([128, kj, 16, 8]),
                        op=AND,
                    )
                    oh01 = wpool.tile([128, kj * 128], bf16, tag="oh01")
                    nc.vector.tensor_tensor(
                        out=oh01[:].rearrange("p (j b v) -> p j b v", b=16, v=8),
                        in0=ohtmp[:].rearrange("p (j b v) -> p j b v", b=16, v=8),
                        in1=mask_bc[:].rearrange("p (b v) -> p b v", v=8)
                        .unsqueeze(1).to_broadcast([128, kj, 16, 8]),
                        op=EQ,
                    )

                    # scores: sc = asr + adst (both single bf16)
                    if variant == "a":
                        ohTtmp = wpool.tile([128, kj * 128], i16, tag="ohTtmp")
                        nc.vector.tensor_tensor(
                            out=ohTtmp[:].rearrange("p (j b v) -> p j b v", b=16, v=8),
                            in0=obT_t[:].rearrange("p (j v) -> p j v", v=8)
                            .unsqueeze(2).to_broadcast([128, kj, 16, 8]),
                            in1=mask_bc[:].rearrange("p (b v) -> p b v", v=8)
                            .unsqueeze(1).to_broadcast([128, kj, 16, 8]),
                            op=AND,
                        )
                        ohT01 = wpool.tile([128, kj * 128], bf16, tag="ohT01")
                        nc.vector.tensor_tensor(
                            out=ohT01[:].rearrange("p (j b v) -> p j b v", b=16, v=8),
                            in0=ohTtmp[:].rearrange("p (j b v) -> p j b v", b=16, v=8),
                            in1=mask_bc[:].rearrange("p (b v) -> p b v", v=8)
                            .unsqueeze(1).to_broadcast([128, kj, 16, 8]),
                            op=EQ,
                        )
                        adw = adst_all[:, w * H : (w + 1) * H]
                        adx_ps = p2psb.tile([128, kj * H], f32, tag="adx")
                        for j in range(kj):
                            nc.tensor.matmul(
                                adx_ps[:, j * H : (j + 1) * H],
                                lhsT=ohT01[:, j * 128 : (j + 1) * 128],
                                rhs=adw,
                                start=True,
                                stop=True,
                            )
                        adx = wpool.tile([128, kj * H], f32, tag="adxs")
                        nc.vector.tensor_copy(adx[:], adx_ps[:])
                        adv = adx[:].rearrange("p (j h) -> p j h", h=H)
                    else:
                        adv = gdv[:, :, 0:H]
                    sc = wpool.tile([128, kj * H], f32, tag="sc")
                    nc.vector.tensor_add(
                        sc[:].rearrange("p (j h) -> p j h", h=H),
                        gv[:, :, HD : HD + H],
                        adv,
                    )
                    # exp(leaky_relu(s)) = max(exp(s), exp(0.2*s)) (monotone)
                    e1 = wpool.tile([128, kj * H], f32, tag="e1")
                    nc.scalar.activation(
                        e1[:], sc[:], mybir.ActivationFunctionType.Exp,
                    )
                    e2 = wpool.tile([128, kj * H], f32, tag="e2")
                    nc.scalar.activation(
                        e2[:], sc[:], mybir.ActivationFunctionType.Exp,
                        scale=NEG_SLOPE,
                    )
                    # messages mv = [e*h(256, (d,h)-major) | e(4)] bf16/subtile
                    MC = HD + 2 * H  # 264-col pitch (16B aligned); 260 used
                    mv = wpool.tile([128, kj * MC], bf16, tag="mv")
                    mvv = mv[:].rearrange("p (j c) -> p j c", c=MC)
                    # the max writes e directly into mv cols 256:260 (bf16)
                    nc.vector.tensor_tensor(
                        out=mvv[:, :, HD : HD + H],
                        in0=e1[:].rearrange("p (j h) -> p j h", h=H),
                        in1=e2[:].rearrange("p (j h) -> p j h", h=H),
                        op=MAX,
                    )
                    nc.vector.tensor_mul(
                        mvv[:, :, 0:HD].rearrange("p j (d h) -> p j d h", h=H),
                        gv[:, :, 0:HD].rearrange("p j (d h) -> p j d h", h=H),
                        mvv[:, :, HD : HD + H].unsqueeze(2).to_broadcast(
                            [128, kj, D, H]
                        ),
                    )
                    # segment-sum: accdns[slot, 0:256]=sum e*h, [256:260]=sum e
                    accdns = p2ps.tile([128, HD + H], f32, tag="accdns")
                    for j in range(kj):
                        nc.tensor.matmul(
                            accdns[:],
                            lhsT=oh01[:, j * 128 : (j + 1) * 128],
                            rhs=mvv[:, j, 0 : HD + H],
                            start=(j == 0),
                            stop=(j == kj - 1),
                        )
                    # finalize: out = acc / (dns + eps) + bias  ((d,h)->(h,d))
                    acc_sb = fpool.tile([128, HD + H], f32, tag="acc_sb")
                    nc.vector.tensor_copy(acc_sb[:], accdns[:])
                    dnse = fpool.tile([128, H], f32, tag="dnse")
                    nc.vector.tensor_scalar_add(dnse[:], acc_sb[:, HD : HD + H], EPS)
                    dnr = fpool.tile([128, H], f32, tag="dnr")
                    nc.vector.reciprocal(dnr[:], dnse[:])
                    outw = fpool.tile([128, HD], f32, tag="outw")
                    nc.vector.tensor_mul(
                        outw[:].rearrange("p (h d) -> p h d", d=D),
                        acc_sb[:, 0:HD].rearrange("p (d h) -> p h d", h=H),
                        dnr[:].unsqueeze(-1).to_broadcast([128, H, D]),
                    )
                    nc.vector.tensor_add(outw[:], outw[:], bias_bc[:])
                    nc.scalar.dma_start(out_d[w * 128 : (w + 1) * 128, :], outw[:])
    nc.compile()
    # compile()'s late passes (act-table loads, hostgen rebases) can leave
    # >1-wait instructions behind; one more split pass clears them (the TRN2
    # ISA allows a single sem wait per compute instruction).
    nc.generate_event_semaphores()
    return nc


def _make_in_maps(x, W, att_src, att_dst, bias, hp, variant="b"):
    """Per-core input dicts from host-prep results `hp`."""
    import ml_dtypes

    bf16 = ml_dtypes.bfloat16
    x = np.asarray(x, dtype=np.float32)
    xT = np.zeros((IN_DIM, NROWS_ALL), dtype=bf16)
    xT[:, :N] = x.T.astype(bf16)
    xT = np.ascontiguousarray(xT)
    # W and att rows are sent (d,h)-major so the table rows (and phase-2
    # message columns) are (d,h)-ordered -- keeps the e-broadcast multiply
    # stride-1 in its last dim.  bias stays (h,d) (applied after permute-back).
    W = np.asarray(W, np.float32)
    W_dh = np.ascontiguousarray(W.reshape(IN_DIM, H, D).transpose(0, 2, 1).reshape(IN_DIM, HD))
    asrc_row = np.ascontiguousarray(
        np.asarray(att_src, np.float32).T.reshape(1, HD))  # [D,H] flat
    adst_row = np.ascontiguousarray(
        np.asarray(att_dst, np.float32).T.reshape(1, HD))
    bias_row = np.ascontiguousarray(np.asarray(bias, np.float32).reshape(1, HD))

    in_maps = []
    for c in range(NCORES):
        xdT = np.zeros((IN_DIM, WROWS), dtype=bf16)
        xdT[:, :NPC] = x[c * NPC : (c + 1) * NPC].T.astype(bf16)
        m = {
            "xT": xT,
            "xdstT": np.ascontiguousarray(xdT),
            "W": W_dh,
            "att_src": asrc_row,
            "att_dst": adst_row,
            "bias": bias_row,
            "ilow": np.ascontiguousarray(hp["ilow"][c]),
            "ihigh": np.ascontiguousarray(hp["ihigh"][c]),
            "ohbits": np.ascontiguousarray(hp["ohbits"][c]),
            "mask16": hp["mask16"],
        }
        if variant == "a":
            m["ohTbits"] = np.ascontiguousarray(hp["ohTbits"][c])
        else:
            m["idst"] = np.ascontiguousarray(hp["idst"][c])
        in_maps.append(m)
    return in_maps


VARIANT = "a"


def kernel(x, edge_index, W, att_src, att_dst, bias):
    global LAST_RESULTS
    from concourse.bass_utils import run_bass_kernel_spmd

    edge_index = np.asarray(edge_index)
    hp = _prep_host(edge_index, variant=VARIANT)
    nc = _build_program(hp["KL"], hp["KH"], variant=VARIANT)
    in_maps = _make_in_maps(x, W, att_src, att_dst, bias, hp, variant=VARIANT)

    res = run_bass_kernel_spmd(nc, in_maps, list(range(NCORES)))
    LAST_RESULTS = res

    out = np.empty((N, HD), dtype=np.float32)
    for c in range(NCORES):
        out[c * NPC : (c + 1) * NPC] = res.results[c]["out"][:NPC]
    return out



# revision 4
# speedup vs baseline: 1.5123x; 1.5123x over previous
"""GAT layer (PyG-style, add_self_loops=True) on 8 Trainium2 NeuronCores.

Strategy (per sharding hint): partition destination nodes (and their incident
edges) across the 8 cores; each core owns a contiguous range of 6250 dst nodes.

Per core:
  phase 1: full projection table row[n] = [h(256) | asr(4)] in bf16 (768-B
           pitch, 520 B written) in local DRAM -- replicated compute, zero
           cross-core communication.  Split into TWO tables (lo: nodes <
           25088, hi: rest) because dma_gather indices are int16.
           All projection matmuls run in bf16 (x is pre-cast on host).
  phase 1b: a_dst (bf16) for the core's own 6272 dst nodes, kept in SBUF
           (variant a) or written as a compact 256-B-row DRAM table for a
           third gather (variant b).
  phase 2 (variant a, default): per window of 128 dst nodes, two dma_gathers
           (lo/hi) pull the 768-B source rows for all incident edges.  The
           edge->dst-slot one-hot (and its transpose) arrive from the host as
           packed int16 BITMASKS in bit-plane order and are expanded to bf16
           0/1 matrices with whole-window DVE ops (bitwise_and + is_equal,
           all APs stride-1 so the DVE 2x mode engages) -- no PE transposes.
           a_dst per edge = kj tiny bf16 matmuls (lhsT = one-hot transpose,
           rhs = adst).  scores: sc = asr + adx (both single bf16 -- the
           hi/lo split-precision was dropped, measured error budget allows
           it); e = max(exp(sc), exp(0.2*sc)) (== exp(leaky_relu(sc)),
           exp on Act with its scale port, max on DVE writing bf16 straight
           into the message tile).  messages mv = [e*h | e] bf16 with the
           table in (d,h)-major layout so the e-broadcast multiply stays
           stride-1; segment-sum = kj bf16 matmuls accumulating into one
           PSUM tile; finalize on DVE, out-DMA on the Act HWDGE ring (keeps
           the SP ring a pure load stream).  Gathers for window w+2 are
           issued before computing window w, and the lo-table gathers begin
           while the hi half of the table is still being built.
           Softmax max-subtraction is skipped (shift-invariant, scores O(1)).

Pad edges point at a dummy table row AND have all-zero one-hot bits, so they
contribute exactly nothing.

Host does only index-space work (self-loop append, dst sort, windowing,
padding, int16 index wrapping, one-hot bit packing) plus data layout
(x transposed + cast bf16).
"""

import math

import numpy as np

N = 50000
IN_DIM = 64
H = 4
D = 64
HD = H * D  # 256
ROWC = 384  # bf16 table row pitch: h(256) | asr(4) | pad
WCOLS = HD + H  # 260 cols actually written per table row: h | asr
NEG_SLOPE = 0.2
EPS = 1e-16

NCORES = 8
NPC = N // NCORES  # 6250 dst nodes per core
NWIN = math.ceil(NPC / 128)  # 49 windows
WROWS = NWIN * 128  # 6272
NT1 = 392  # phase-1 tiles (50176 nodes incl. pad)
NROWS_ALL = NT1 * 128  # 50176
SPLIT_T = 196  # lo/hi table split, in 128-row tiles
SPLIT = SPLIT_T * 128  # 25088
LO_TILES = SPLIT_T + 1  # +1 dummy tile
LO_ROWS = LO_TILES * 128  # 25216
HI_TILES = NT1 - SPLIT_T  # 196
HI_ROWS = HI_TILES * 128  # 25088
DUMMY_LO = SPLIT  # row 25088 of lo table (dedicated dummy row)
DUMMY_HI = N - SPLIT  # row 24912 of hi table (= node 50000, h == 0)
B1 = 7  # phase-1 tiles per iteration (divides both 196 and 392)
DSTC = 128  # bf16 cols per tbl_dst row (256 B): adst(4) | pad

LAST_RESULTS = None  # BassKernelResults of the most recent run (for test.py)


def _wrap_idx(ids):
    """[n] int -> dma_gather wrapped layout [128, n/16] int16
    (idx i at [i%16, i//16], replicated across the 8 Q7 core groups)."""
    n = len(ids)
    w16 = ids.reshape(n // 16, 16).T.astype(np.int16)  # [16, n/16]
    return np.tile(w16, (8, 1))


def _prep_host(edge_index, variant="b"):
    """Returns dict of per-core host tensors + (KL, KH)."""
    src = np.concatenate([edge_index[0], np.arange(N, dtype=np.int64)]).astype(np.int64)
    dst = np.concatenate([edge_index[1], np.arange(N, dtype=np.int64)]).astype(np.int64)
    order = np.argsort(dst, kind="stable")
    src = src[order].astype(np.int32)
    dst = dst[order].astype(np.int32)

    bounds = [c * NPC + w * 128 for c in range(NCORES) for w in range(NWIN)]
    bounds.append(N)
    cuts = np.searchsorted(dst, np.asarray(bounds))

    lo_counts = np.zeros(NCORES * NWIN, np.int64)
    hi_counts = np.zeros(NCORES * NWIN, np.int64)
    for b in range(NCORES * NWIN):
        s = src[cuts[b] : cuts[b + 1]]
        lo_counts[b] = int((s < SPLIT).sum())
        hi_counts[b] = len(s) - lo_counts[b]
    KL = max(1, math.ceil(lo_counts.max() / 128))
    KH = max(1, math.ceil(hi_counts.max() / 128))
    kj = KL + KH

    # windows 0..1 gather the full padded lists (dummy rows) so both g
    # buffers are fully initialized; later windows use -1 trailing pads +
    # true counts, and their skipped regions read the previous window's
    # (finite) data, which the all-zero one-hot columns nullify.
    ilow = np.full((NCORES, NWIN, KL * 128), DUMMY_LO, np.int32)
    ihigh = np.full((NCORES, NWIN, KH * 128), DUMMY_HI, np.int32)
    idst = np.zeros((NCORES, NWIN, kj * 128), np.int32)  # pad -> row 0
    ohbits = np.zeros((NCORES, NWIN, 128, kj * 8), np.uint16)
    ohTbits = np.zeros((NCORES, NWIN, 128, kj * 8), np.uint16)
    for c in range(NCORES):
        base = c * NPC
        for w in range(NWIN):
            b = c * NWIN + w
            s = src[cuts[b] : cuts[b + 1]]
            d = dst[cuts[b] : cuts[b + 1]] - base - w * 128
            m = s < SPLIT
            slo, dlo = s[m], d[m]
            shi, dhi = s[~m] - SPLIT, d[~m]
            # ascending source rows => HBM page locality in the gather
            o = np.argsort(slo, kind="stable")
            slo, dlo = slo[o], dlo[o]
            o = np.argsort(shi, kind="stable")
            shi, dhi = shi[o], dhi[o]
            ilow[c, w, : len(slo)] = slo
            ihigh[c, w, : len(shi)] = shi
            # negpad skip disabled: full padded gathers (dummy rows)
            # edge position in gather output: idx i -> (part i%128, subtile i//128)
            nl, nh = len(slo), len(shi)
            i = np.arange(nl)
            jl, pl = i // 128, i % 128
            i = np.arange(nh)
            jh, ph = KL + i // 128, i % 128
            jj = np.concatenate([jl, jh])
            pp = np.concatenate([pl, ph])
            ss = np.concatenate([dlo, dhi])  # dst slot per edge
            # local dst id for the a_dst gather (variant b)
            idst[c, w, jj * 128 + pp] = w * 128 + ss
            # bit-plane packing (keeps device-side expansion APs stride-1):
            # oh[p, j*128+slot]: bit (slot//8) of word [p, j*8 + slot%8]
            np.bitwise_or.at(
                ohbits[c, w], (pp, jj * 8 + ss % 8),
                (np.uint16(1) << (ss // 8).astype(np.uint16)),
            )
            # ohT[slot, j*128+p]: bit (p//8) of word [slot, j*8 + p%8]
            np.bitwise_or.at(
                ohTbits[c, w], (ss, jj * 8 + pp % 8),
                (np.uint16(1) << (pp // 8).astype(np.uint16)),
            )
    ilow_w = np.zeros((NCORES, NWIN, 128, KL * 8), np.int16)
    ihigh_w = np.zeros((NCORES, NWIN, 128, KH * 8), np.int16)
    idst_w = np.zeros((NCORES, NWIN, 128, kj * 8), np.int16)
    for c in range(NCORES):
        for w in range(NWIN):
            ilow_w[c, w] = _wrap_idx(ilow[c, w])
            ihigh_w[c, w] = _wrap_idx(ihigh[c, w])
            idst_w[c, w] = _wrap_idx(idst[c, w])
    # mask128[p, b*8+w] = 1 << b  (bit-plane expansion constant)
    mrow = (np.uint16(1) << (np.arange(128, dtype=np.uint16) // 8))
    mask128 = np.tile(mrow[None, :], (128, 1)).view(np.int16)
    return {
        "ilow": ilow_w,
        "ihigh": ihigh_w,
        "idst": idst_w,
        "ohbits": ohbits.view(np.int16),
        "ohTbits": ohTbits.view(np.int16),
        "mask16": mask128,
        "KL": KL,
        "KH": KH,
    }


def _build_program(KL, KH, variant="b", ablate="full"):
    import concourse.bass as bass
    import concourse.bacc as bacc
    import concourse.tile as tile
    from concourse import mybir

    f32 = mybir.dt.float32
    bf16 = mybir.dt.bfloat16
    i16 = mybir.dt.int16
    kj = KL + KH

    nc = bacc.Bacc(None, target_bir_lowering=False, num_swdge_queues=4)

    xT_d = nc.dram_tensor("xT", [IN_DIM, NROWS_ALL], bf16, kind="ExternalInput")
    xdT_d = nc.dram_tensor("xdstT", [IN_DIM, WROWS], bf16, kind="ExternalInput")
    W_d = nc.dram_tensor("W", [IN_DIM, HD], f32, kind="ExternalInput")
    asrc_d = nc.dram_tensor("att_src", [1, HD], f32, kind="ExternalInput")
    adst_d = nc.dram_tensor("att_dst", [1, HD], f32, kind="ExternalInput")
    bias_d = nc.dram_tensor("bias", [1, HD], f32, kind="ExternalInput")
    il_d = nc.dram_tensor("ilow", [NWIN, 128, KL * 8], i16, kind="ExternalInput")
    ih_d = nc.dram_tensor("ihigh", [NWIN, 128, KH * 8], i16, kind="ExternalInput")
    ob_d = nc.dram_tensor("ohbits", [NWIN, 128, kj * 8], i16, kind="ExternalInput")
    if variant == "a":
        obT_d = nc.dram_tensor("ohTbits", [NWIN, 128, kj * 8], i16, kind="ExternalInput")
    else:
        id_d = nc.dram_tensor("idst", [NWIN, 128, kj * 8], i16, kind="ExternalInput")
    mask_d = nc.dram_tensor("mask16", [128, 128], i16, kind="ExternalInput")
    out_d = nc.dram_tensor("out", [WROWS, HD], f32, kind="ExternalOutput")
    tbl_lo = nc.dram_tensor("tbl_lo", [LO_ROWS, ROWC], bf16)  # 768 B pitch
    tbl_hi = nc.dram_tensor("tbl_hi", [HI_ROWS, ROWC], bf16)
    if variant == "b":
        tbl_dst = nc.dram_tensor("tbl_dst", [WROWS, DSTC], bf16)  # 256 B rows

    X = mybir.AxisListType.X
    EQ = mybir.AluOpType.is_equal
    AND = mybir.AluOpType.bitwise_and
    MULT = mybir.AluOpType.mult
    MAX = mybir.AluOpType.max

    with tile.TileContext(nc) as tc:
        with tc.tile_pool(name="const", bufs=1) as cpool:
            spsum_cm = tc.tile_pool(name="setup_psum", bufs=1, space="PSUM")
            spsum = spsum_cm.__enter__()
            ones = cpool.tile([1, 128], f32)
            nc.vector.memset(ones[:], 1.0)

            # WA = [W | Wsrc | Wdst], Wsrc[k,h] = sum_d W[k,h*D+d]*att_src[h,d]
            # built in f32, then cast to bf16 for the phase-1 matmuls.
            WACOLS = HD + 2 * H  # Wdst block feeds phase-1b only
            wa_tmp = cpool.tile([IN_DIM, WACOLS], f32)
            nc.vector.memset(wa_tmp[:], 0.0)
            nc.sync.dma_start(wa_tmp[:, 0:HD], W_d[:, :])
            att_s_raw = cpool.tile([1, HD], f32)
            nc.sync.dma_start(att_s_raw[:], asrc_d[:, :])
            att_t_raw = cpool.tile([1, HD], f32)
            nc.sync.dma_start(att_t_raw[:], adst_d[:, :])
            att_s = cpool.tile([1, HD], f32)
            nc.vector.tensor_copy(att_s[:], att_s_raw[:])
            att_t = cpool.tile([1, HD], f32)
            nc.vector.tensor_copy(att_t[:], att_t_raw[:])
            for att_tile, col0 in ((att_s, HD), (att_t, HD + H)):
                attb = spsum.tile([IN_DIM, HD], f32, tag="attb")
                nc.tensor.matmul(
                    attb[:], lhsT=ones[:1, 0:IN_DIM], rhs=att_tile[:],
                    start=True, stop=True,
                )
                tmp = cpool.tile([IN_DIM, HD], f32, tag="tmp")
                nc.vector.tensor_mul(tmp[:], wa_tmp[:, 0:HD], attb[:])
                # W and att rows arrive (d,h)-major; reduce over d
                nc.vector.reduce_sum(
                    out=wa_tmp[:, col0 : col0 + H],
                    in_=tmp[:].rearrange("k (d h) -> k h d", h=H),
                    axis=X,
                )
            WA = cpool.tile([IN_DIM, WACOLS], bf16)
            nc.vector.tensor_copy(WA[:], wa_tmp[:])

            bias_raw = cpool.tile([1, HD], f32)
            nc.sync.dma_start(bias_raw[:], bias_d[:, :])
            bias_sb = cpool.tile([1, HD], f32)
            nc.vector.tensor_copy(bias_sb[:], bias_raw[:])
            bb = spsum.tile([128, HD], f32)
            nc.tensor.matmul(bb[:], lhsT=ones[:1, :], rhs=bias_sb[:], start=True, stop=True)
            bias_bc = cpool.tile([128, HD], f32)
            nc.scalar.copy(bias_bc[:], bb[:])

            mask_bc = cpool.tile([128, 128], i16)
            mask_raw = cpool.tile([128, 128], i16, tag="mask_raw")
            nc.sync.dma_start(mask_raw[:], mask_d[:, :])
            nc.vector.tensor_copy(mask_bc[:], mask_raw[:])

            # a_dst (bf16) for the core's own dst shard: [128, NWIN*4]
            adst_all = cpool.tile([128, NWIN * H], bf16)

            spsum_cm.__exit__(None, None, None)  # free setup PSUM banks

            # ---------------- phases (unified pools, pipelined) ----------------
            with (
                tc.tile_pool(name="p1", bufs=4) as p1,
                tc.tile_pool(name="p1ps", bufs=3, space="PSUM") as p1ps,
                tc.tile_pool(name="p1psb", bufs=1, space="PSUM") as p1psb,
                tc.tile_pool(name="gat", bufs=2) as gpool,
                tc.tile_pool(name="edg", bufs=3) as epool,
                tc.tile_pool(name="wrk", bufs=2) as wpool,
                tc.tile_pool(name="fin", bufs=2) as fpool,
                tc.tile_pool(name="p2ps", bufs=3 if variant == "b" else 2,
                             space="PSUM") as p2ps,
                tc.tile_pool(name="p2psb", bufs=2, space="PSUM") as p2psb,
            ):
                # phase 1b FIRST: a_dst (hi|lo bf16) for own dst shard, so the
                # compact dst table is ready before any dst-gather fires
                for w in range(NWIN):
                    xd = p1.tile([IN_DIM, 128], bf16, tag="xdr")
                    nc.sync.dma_start(xd[:], xdT_d[:, w * 128 : (w + 1) * 128])
                    adp = p1psb.tile([128, H], f32, tag="adp")
                    nc.tensor.matmul(
                        adp[:], lhsT=xd[:], rhs=WA[:, HD + H : HD + 2 * H],
                        start=True, stop=True,
                    )
                    c0 = w * H
                    nc.vector.tensor_copy(adst_all[:, c0 : c0 + H], adp[:])
                if variant == "b":
                    nc.scalar.dma_start(
                        tbl_dst[:, 0:H].rearrange("(w p) c -> p w c", p=128),
                        adst_all[:].rearrange("p (w c) -> p w c", c=H),
                    )

                def p1_tile(it):
                    t0 = it * B1
                    xt = p1.tile([IN_DIM, B1 * 128], bf16, tag="xtr")
                    nc.sync.dma_start(xt[:], xT_d[:, t0 * 128 : (t0 + B1) * 128])
                    hs = p1.tile([128, B1 * WCOLS], bf16, tag="hs")
                    for k in range(B1):
                        hp = p1ps.tile([128, WCOLS], f32, tag="hp")
                        nc.tensor.matmul(
                            hp[:],
                            lhsT=xt[:, k * 128 : (k + 1) * 128],
                            rhs=WA[:, 0:WCOLS],
                            start=True,
                            stop=True,
                        )
                        # cast h+asr to bf16, split across DVE and Act
                        cast = nc.vector.tensor_copy if k < 3 else nc.scalar.copy
                        cast(hs[:, k * WCOLS : (k + 1) * WCOLS], hp[:])
                    if t0 < SPLIT_T:
                        dst_ap = tbl_lo[t0 * 128 : (t0 + B1) * 128, 0:WCOLS]
                    else:
                        u = t0 - SPLIT_T
                        dst_ap = tbl_hi[u * 128 : (u + B1) * 128, 0:WCOLS]
                    nc.scalar.dma_start(
                        dst_ap.rearrange("(k p) c -> p k c", p=128),
                        hs[:].rearrange("p (k c) -> p k c", c=WCOLS),
                    )

                # lo half of the src table
                for it in range(SPLIT_T // B1):
                    p1_tile(it)
                # dummy rows of the lo table (gather pads point here)
                zrow = cpool.tile([128, WCOLS], bf16)
                nc.vector.memset(zrow[:], 0.0)
                nc.vector.memset(zrow[:, HD : HD + H], -1e30)  # asr col
                nc.sync.dma_start(tbl_lo[DUMMY_LO : DUMMY_LO + 128, 0:WCOLS], zrow[:])

                # zero both g buffers once: rows skipped by the negative-pad
                # gathers then read as 0.0 (finite) forever after
                for _ in range(2):
                    gz = gpool.tile([128, kj * ROWC], bf16, tag="g")
                    nc.vector.memset(gz[:], 0.0)

                def issue_pre(w):
                    """idx DMAs + lo/dst gathers: only needs tbl_lo/tbl_dst,
                    so these run while the hi table is still being built."""
                    il_t = epool.tile([128, KL * 8], i16, tag="il")
                    nc.sync.dma_start(il_t[:], il_d[w])
                    ih_t = epool.tile([128, KH * 8], i16, tag="ih")
                    nc.sync.dma_start(ih_t[:], ih_d[w])
                    ob_t = epool.tile([128, kj * 8], i16, tag="ob")
                    nc.sync.dma_start(ob_t[:], ob_d[w])
                    g = gpool.tile([128, kj * ROWC], bf16, tag="g")
                    gv = g[:].rearrange("p (j c) -> p j c", c=ROWC)
                    nc.gpsimd.dma_gather(
                        out_ap=gv[:, 0:KL, :], in_ap=tbl_lo[:, :], idxs_ap=il_t[:],
                        num_idxs=KL * 128, num_idxs_reg=KL * 128, elem_size=ROWC,
                        single_packet=False, queue_num=(2 * w) % 4,
                    )
                    st = {"g": g, "gv": gv, "ih": ih_t, "ob": ob_t,
                          "gdv": None, "obT": None}
                    if variant == "a":
                        obT_t = epool.tile([128, kj * 8], i16, tag="obT")
                        nc.sync.dma_start(obT_t[:], obT_d[w])
                        st["obT"] = obT_t
                    else:
                        id_t = epool.tile([128, kj * 8], i16, tag="idt")
                        nc.sync.dma_start(id_t[:], id_d[w])
                        gd = gpool.tile([128, kj * DSTC], bf16, tag="gd")
                        st["gdv"] = gd[:].rearrange("p (j c) -> p j c", c=DSTC)
                        nc.gpsimd.dma_gather(
                            out_ap=st["gdv"][:, :, :], in_ap=tbl_dst[:, :],
                            idxs_ap=id_t[:],
                            num_idxs=kj * 128, num_idxs_reg=kj * 128,
                            elem_size=DSTC, single_packet=False,
                        )
                    return st

                def issue_hi(st, w):
                    nc.gpsimd.dma_gather(
                        out_ap=st["gv"][:, KL:kj, :], in_ap=tbl_hi[:, :],
                        idxs_ap=st["ih"][:],
                        num_idxs=KH * 128, num_idxs_reg=KH * 128, elem_size=ROWC,
                        single_packet=False, queue_num=(2 * w + 1) % 4,
                    )

                nwin2 = NWIN if ablate != "p1" else 0
                if nwin2:
                    pend = [issue_pre(0), issue_pre(1) if nwin2 > 1 else None]

                # hi half of the src table (lo/dst gathers overlap this)
                for it in range(SPLIT_T // B1, NT1 // B1):
                    p1_tile(it)
                nc.sync.dma_start(
                    tbl_hi[DUMMY_HI : DUMMY_HI + 1, HD : HD + H],
                    zrow[:1, HD : HD + H],
                )

                if ablate == "p1":
                    zo = fpool.tile([128, HD], f32)
                    nc.vector.memset(zo[:], 0.0)
                    for w in range(NWIN):
                        nc.sync.dma_start(out_d[w * 128 : (w + 1) * 128, :], zo[:])

                for w in range(nwin2):
                    st = pend[0]
                    issue_hi(st, w)
                    pend[0] = pend[1]
                    if w + 2 < nwin2:
                        pend[1] = issue_pre(w + 2)
                    gv, gdv, ob_t, obT_t = st["gv"], st["gdv"], st["ob"], st["obT"]
                    g = st["g"]

                    if ablate == "p1g":
                        outw = fpool.tile([128, HD], f32, tag="outw")
                        nc.vector.tensor_copy(outw[:], g[:, 0:HD])
                        nc.scalar.dma_start(out_d[w * 128 : (w + 1) * 128, :], outw[:])
                        continue

                    # one-hot expand (bit-plane): bits -> bf16 0/1 [128, kj*128]
                    # col j*128 + b*8 + v <=> slot b*8+v; all APs stride-1 last
                    ohtmp = wpool.tile([128, kj * 128], i16, tag="ohtmp")
                    nc.vector.tensor_tensor(
                        out=ohtmp[:].rearrange("p (j b v) -> p j b v", b=16, v=8),
                        in0=ob_t[:].rearrange("p (j v) -> p j v", v=8)
                        .unsqueeze(2).to_broadcast([128, kj, 16, 8]),
                        in1=mask_bc[:].rearrange("p (b v) -> p b v", v=8)
                        .unsqueeze(1).to_broadcast([128, kj, 16, 8]),
                        op=AND,
                    )
                    oh01 = wpool.tile([128, kj * 128], bf16, tag="oh01")
                    nc.vector.tensor_tensor(
                        out=oh01[:].rearrange("p (j b v) -> p j b v", b=16, v=8),
                        in0=ohtmp[:].rearrange("p (j b v) -> p j b v", b=16, v=8),
                        in1=mask_bc[:].rearrange("p (b v) -> p b v", v=8)
                        .unsqueeze(1).to_broadcast([128, kj, 16, 8]),
                        op=EQ,
                    )

                    # scores: sc = asr + adst (both single bf16)
                    if variant == "a":
                        ohTtmp = wpool.tile([128, kj * 128], i16, tag="ohTtmp")
                        nc.vector.tensor_tensor(
                            out=ohTtmp[:].rearrange("p (j b v) -> p j b v", b=16, v=8),
                            in0=obT_t[:].rearrange("p (j v) -> p j v", v=8)
                            .unsqueeze(2).to_broadcast([128, kj, 16, 8]),
                            in1=mask_bc[:].rearrange("p (b v) -> p b v", v=8)
                            .unsqueeze(1).to_broadcast([128, kj, 16, 8]),
                            op=AND,
                        )
                        ohT01 = wpool.tile([128, kj * 128], bf16, tag="ohT01")
                        nc.vector.tensor_tensor(
                            out=ohT01[:].rearrange("p (j b v) -> p j b v", b=16, v=8),
                            in0=ohTtmp[:].rearrange("p (j b v) -> p j b v", b=16, v=8),
                            in1=mask_bc[:].rearrange("p (b v) -> p b v", v=8)
                            .unsqueeze(1).to_broadcast([128, kj, 16, 8]),
                            op=EQ,
                        )
                        adw = adst_all[:, w * H : (w + 1) * H]
                        adx_ps = p2psb.tile([128, kj * H], f32, tag="adx")
                        for j in range(kj):
                            nc.tensor.matmul(
                                adx_ps[:, j * H : (j + 1) * H],
                                lhsT=ohT01[:, j * 128 : (j + 1) * 128],
                                rhs=adw,
                                start=True,
                                stop=True,
                            )
                        adx = wpool.tile([128, kj * H], f32, tag="adxs")
                        nc.vector.tensor_copy(adx[:], adx_ps[:])
                        adv = adx[:].rearrange("p (j h) -> p j h", h=H)
                    else:
                        adv = gdv[:, :, 0:H]
                    sc = wpool.tile([128, kj * H], f32, tag="sc")
                    nc.vector.tensor_add(
                        sc[:].rearrange("p (j h) -> p j h", h=H),
                        gv[:, :, HD : HD + H],
                        adv,
                    )
                    # exp(leaky_relu(s)) = max(exp(s), exp(0.2*s)) (monotone)
                    e1 = wpool.tile([128, kj * H], f32, tag="e1")
                    nc.scalar.activation(
                        e1[:], sc[:], mybir.ActivationFunctionType.Exp,
                    )
                    e2 = wpool.tile([128, kj * H], f32, tag="e2")
                    nc.scalar.activation(
                        e2[:], sc[:], mybir.ActivationFunctionType.Exp,
                        scale=NEG_SLOPE,
                    )
                    # messages mv = [e*h(256, (d,h)-major) | e(4)] bf16/subtile
                    MC = HD + 2 * H  # 264-col pitch (16B aligned); 260 used
                    mv = wpool.tile([128, kj * MC], bf16, tag="mv")
                    mvv = mv[:].rearrange("p (j c) -> p j c", c=MC)
                    # the max writes e directly into mv cols 256:260 (bf16)
                    nc.vector.tensor_tensor(
                        out=mvv[:, :, HD : HD + H],
                        in0=e1[:].rearrange("p (j h) -> p j h", h=H),
                        in1=e2[:].rearrange("p (j h) -> p j h", h=H),
                        op=MAX,
                    )
                    nc.vector.tensor_mul(
                        mvv[:, :, 0:HD].rearrange("p j (d h) -> p j d h", h=H),
                        gv[:, :, 0:HD].rearrange("p j (d h) -> p j d h", h=H),
                        mvv[:, :, HD : HD + H].unsqueeze(2).to_broadcast(
                            [128, kj, D, H]
                        ),
                    )
                    # segment-sum: accdns[slot, 0:256]=sum e*h, [256:260]=sum e
                    accdns = p2ps.tile([128, HD + H], f32, tag="accdns")
                    for j in range(kj):
                        nc.tensor.matmul(
                            accdns[:],
                            lhsT=oh01[:, j * 128 : (j + 1) * 128],
                            rhs=mvv[:, j, 0 : HD + H],
                            start=(j == 0),
                            stop=(j == kj - 1),
                        )
                    # finalize: out = acc / (dns + eps) + bias  ((d,h)->(h,d))
                    acc_sb = fpool.tile([128, HD + H], f32, tag="acc_sb")
                    nc.vector.tensor_copy(acc_sb[:], accdns[:])
                    dnse = fpool.tile([128, H], f32, tag="dnse")
                    nc.vector.tensor_scalar_add(dnse[:], acc_sb[:, HD : HD + H], EPS)
                    dnr = fpool.tile([128, H], f32, tag="dnr")
                    nc.vector.reciprocal(dnr[:], dnse[:])
                    outw = fpool.tile([128, HD], f32, tag="outw")
                    nc.vector.tensor_mul(
                        outw[:].rearrange("p (h d) -> p h d", d=D),
                        acc_sb[:, 0:HD].rearrange("p (d h) -> p h d", h=H),
                        dnr[:].unsqueeze(-1).to_broadcast([128, H, D]),
                    )
                    nc.vector.tensor_add(outw[:], outw[:], bias_bc[:])
                    nc.scalar.dma_start(out_d[w * 128 : (w + 1) * 128, :], outw[:])
    nc.compile()
    # compile()'s late passes (act-table loads, hostgen rebases) can leave
    # >1-wait instructions behind; one more split pass clears them (the TRN2
    # ISA allows a single sem wait per compute instruction).
    nc.generate_event_semaphores()
    return nc


def _make_in_maps(x, W, att_src, att_dst, bias, hp, variant="b"):
    """Per-core input dicts from host-prep results `hp`."""
    import ml_dtypes

    bf16 = ml_dtypes.bfloat16
    x = np.asarray(x, dtype=np.float32)
    xT = np.zeros((IN_DIM, NROWS_ALL), dtype=bf16)
    xT[:, :N] = x.T.astype(bf16)
    xT = np.ascontiguousarray(xT)
    # W and att rows are sent (d,h)-major so the table rows (and phase-2
    # message columns) are (d,h)-ordered -- keeps the e-broadcast multiply
    # stride-1 in its last dim.  bias stays (h,d) (applied after permute-back).
    W = np.asarray(W, np.float32)
    W_dh = np.ascontiguousarray(W.reshape(IN_DIM, H, D).transpose(0, 2, 1).reshape(IN_DIM, HD))
    asrc_row = np.ascontiguousarray(
        np.asarray(att_src, np.float32).T.reshape(1, HD))  # [D,H] flat
    adst_row = np.ascontiguousarray(
        np.asarray(att_dst, np.float32).T.reshape(1, HD))
    bias_row = np.ascontiguousarray(np.asarray(bias, np.float32).reshape(1, HD))

    in_maps = []
    for c in range(NCORES):
        xdT = np.zeros((IN_DIM, WROWS), dtype=bf16)
        xdT[:, :NPC] = x[c * NPC : (c + 1) * NPC].T.astype(bf16)
        m = {
            "xT": xT,
            "xdstT": np.ascontiguousarray(xdT),
            "W": W_dh,
            "att_src": asrc_row,
            "att_dst": adst_row,
            "bias": bias_row,
            "ilow": np.ascontiguousarray(hp["ilow"][c]),
            "ihigh": np.ascontiguousarray(hp["ihigh"][c]),
            "ohbits": np.ascontiguousarray(hp["ohbits"][c]),
            "mask16": hp["mask16"],
        }
        if variant == "a":
            m["ohTbits"] = np.ascontiguousarray(hp["ohTbits"][c])
        else:
            m["idst"] = np.ascontiguousarray(hp["idst"][c])
        in_maps.append(m)
    return in_maps


VARIANT = "a"


def kernel(x, edge_index, W, att_src, att_dst, bias):
    global LAST_RESULTS
    from concourse.bass_utils import run_bass_kernel_spmd

    edge_index = np.asarray(edge_index)
    hp = _prep_host(edge_index, variant=VARIANT)
    nc = _build_program(hp["KL"], hp["KH"], variant=VARIANT)
    in_maps = _make_in_maps(x, W, att_src, att_dst, bias, hp, variant=VARIANT)

    res = run_bass_kernel_spmd(nc, in_maps, list(range(NCORES)))
    LAST_RESULTS = res

    out = np.empty((N, HD), dtype=np.float32)
    for c in range(NCORES):
        out[c * NPC : (c + 1) * NPC] = res.results[c]["out"][:NPC]
    return out

